# revision 1
# baseline (speedup 1.0000x reference)
"""DigitCapsules routing kernel for 8 Trainium2 NeuronCores.

Strategy: shard the in_capsule dimension (32 -> 4 per core) so each core
reads only its slice of W (the dominant tensor) and u.  Each core computes
its u_sum slice [B, 4, OC, OCH] with TensorE matmuls (bf16 inputs, fp32
accumulate) and derives its u_dot slice [4, OC] directly from u_sum
(u_dot[i,o] = sum_{b,c} u_sum[b,i,o,c]) with one reduce + one ones-matmul.
One AllGather shares all slices and every core runs the tiny 3-iteration
dynamic-routing loop redundantly on the full u_sum.  Core 0's output is
returned.

Layouts: the gathered u_sum lives as U_G[p, r, (b c)] with the partition
index p = 16*ia + 4*ib + j encoding (rank ia, local capsule ib, o-group j)
and o = 4*r + j.  The per-o mixing weights are kept directly in the masked
lhsT layout cgz[p, 10*r + o] = c~[i(p), o] * [o == 4*r + j(p)] (c~ =
unnormalized softmax numerator) and updated multiplicatively each routing
iteration: cgz *= exp(u_dot[i(p), o'] * w[o']), where the rank-1 argument
is produced by a single k=10 matmul.  1/sumexp rides the post-matmul
per-partition scalars.

Self-contained: hardcodes all shapes; only imports installed packages.
"""

import numpy as np

# problem shapes
B = 64
IC = 32
ICH = 8
WID = 6
HEI = 6
D = ICH * WID * HEI          # 288
DP = 384                     # D padded to 3 k-chunks of 128
KC = 3                       # k-chunks per contraction
OC = 10
OCH = 16
BC = B * OCH                 # 1024
NUM_ROUTING = 3
NCORES = 8
ICL = IC // NCORES           # 4 local in-capsules per core

# AllGather chunk layout per rank (bf16): (ib:4, jr:12, b:64, c:16) with
# jr = 3*j + r and o = 4*r + j.  Slots with o >= 10 are zero padding that
# the cgz mask ignores, so u_dot (o, i) is stashed inside the (ib=0, jr=8)
# pad slot and the whole chunk stays contiguous (one-DMA gather far side).
USZ = 4 * 4 * 3 * B * OCH    # 49152 bf16 -> 98304 B, 32-aligned
CHUNK = USZ
UD_OFF = 2 * 3 * B * OCH + 2 * B * OCH  # 8192, slot (ib=0, j=2, r=2)

_CACHE = {}


def build(reps=1, single_core=False, no_cc=False):
    """Build + compile the SPMD Bass program (cached per reps).

    reps > 1 repeats the complete pipeline sequentially; used only for
    wall-clock differencing to estimate the on-device execution time.
    single_core=True swaps the AllGather for local DMA copies (same bytes
    moved) so the collective-free program can run under TimelineSim.
    no_cc=True keeps 8 devices but replaces the AllGather with local
    copies (wrong results; used to isolate the collective's HW cost).
    """
    key = ("nc", reps, single_core, no_cc)
    if key in _CACHE:
        return _CACHE[key]

    import concourse.bass as bass  # noqa: F401
    import concourse.mybir as mybir
    from concourse import tile, bacc

    f32 = mybir.dt.float32
    bf16 = mybir.dt.bfloat16
    AX = mybir.AxisListType
    OP = mybir.AluOpType
    ACT = mybir.ActivationFunctionType

    local_cc = single_core or no_cc
    nc = bacc.Bacc("TRN2", target_bir_lowering=False, debug=False,
                   num_devices=1 if single_core else NCORES)

    # inputs pre-arranged on the host so every load is a contiguous DMA
    uw_in = nc.dram_tensor("uw_t", [128, ICL, KC, B + OC * OCH], bf16,
                           kind="ExternalInput")
    # md packs maskm (cols 0:30) and diagm (parts 0:10, cols 30:60)
    md_in = nc.dram_tensor("md", [128, 2 * 3 * OC], bf16,
                           kind="ExternalInput")
    v_out = nc.dram_tensor("v", [B, OC, OCH], f32, kind="ExternalOutput")

    def emit(tc, sb, dram):
        # ---------------- phase 1: local u_sum + u_dot ----------------
        uw_sb = sb.tile([128, ICL, KC, B + OC * OCH], bf16, name="uw_sb")
        nc.sync.dma_start(uw_sb[:, 0:2], uw_in[:, 0:2])
        nc.sync.dma_start(uw_sb[:, 2:4], uw_in[:, 2:4])
        md_sb = sb.tile([128, 2 * 3 * OC], bf16, name="md_sb")
        nc.sync.dma_start(md_sb[:], md_in[:])

        # constants + state built on-chip, off the critical path
        ones_kk = sb.tile([OC, OC], f32, name="ones_kk")
        nc.vector.memset(ones_kk[:], 1.0)
        ones64 = sb.tile([B, 1], f32, name="ones64")
        nc.vector.memset(ones64[:], 1.0)
        # dummy activation pulls LoadActFuncSet under the input-DMA wait
        warm = sb.tile([1, 1], f32, name="warm")
        nc.scalar.activation(warm[:], ones_kk[0:1, 0:1], ACT.Abs)
        bt = sb.tile([OC, NCORES, ICL], f32, name="bt")
        nc.vector.memset(bt[:], 0.0)
        rcp = sb.tile([OC, 1], f32, name="rcp")
        nc.vector.memset(rcp[:], 1.0 / 32.0)  # softmax(0): Z_o = 32

        # local u_sum in chunk layout (b, ib, j, r, c); (j>=2, r=2) zeroed
        s_all = sb.tile([B, ICL, 4, 3, OCH], bf16, name="s_all")
        nc.gpsimd.memset(s_all[:, :, 2:4, 2, :], 0.0)

        q4f = sb.tile([B, OC, ICL], f32, name="q4f")
        cc_in = dram.tile([CHUNK], bf16, name="cc_in")
        cc_sv = cc_in[0:USZ].rearrange("(ib g b c) -> b ib g c",
                                       ib=ICL, g=12, b=B, c=OCH)
        with tc.tile_pool(name="ps1", bufs=4, space="PSUM") as ps1:
            p1s = []
            for pair in ((0, 1), (2, 3)):
                for i in pair:
                    p1 = ps1.tile([B, OC, OCH], f32, name="p1")
                    p1s.append(p1)
                    for k in range(KC):
                        nc.tensor.matmul(
                            p1[:], uw_sb[:, i, k, 0:B], uw_sb[:, i, k, B:],
                            start=(k == 0), stop=(k == KC - 1))
                    # u_sum slice -> s_all columns (j, r) with o = 4r + j
                    nc.vector.tensor_copy(
                        s_all[:, i].rearrange("b j r c -> b r j c")[:, 0:2],
                        p1[:, 0:8, :].rearrange("b (r j) c -> b r j c",
                                                r=2, j=4))
                    nc.scalar.activation(s_all[:, i, 0:2, 2, :],
                                         p1[:, 8:10, :], ACT.Copy)
                # store the finished pair before the q4 reduces so the
                # chunk (and the collective behind it) leaves earlier
                i0 = pair[0]
                nc.sync.dma_start(
                    cc_sv[:, i0:i0 + 2],
                    s_all[:, i0:i0 + 2].rearrange(
                        "b ib j r c -> b ib (j r) c"))
                for i in pair:
                    # q4[b, o, i] = sum_c u_sum (feeds u_dot)
                    nc.vector.tensor_reduce(q4f[:, :, i], p1s[i][:],
                                            axis=AX.X, op=OP.add)
            # u_dot row (o, i): sum_b q4 via fp32 ones-matmul
            udp = ps1.tile([1, OC * ICL], f32, name="udp")
            nc.tensor.matmul(udp[:], ones64[:],
                             q4f[:].rearrange("b o i -> b (o i)"),
                             start=True, stop=True)
            ud_sb = sb.tile([1, OC * ICL], bf16, name="ud_sb")
            nc.scalar.activation(ud_sb[:], udp[:], ACT.Copy)
        nc.sync.dma_start(cc_in[UD_OFF:UD_OFF + ICL * OC], ud_sb[:])

        # ---------------- AllGather ----------------
        cc_out = dram.tile([NCORES * CHUNK], bf16,
                           addr_space="Local" if local_cc else "Shared",
                           name="cc_out")
        if local_cc:
            for ia in range(NCORES):
                nc.sync.dma_start(
                    cc_out[ia * CHUNK:(ia + 1) * CHUNK], cc_in[:])
        else:
            nc.gpsimd.collective_compute(
                "AllGather", OP.bypass,
                replica_groups=[list(range(NCORES))],
                ins=[cc_in[:]], outs=[cc_out[:]],
            )

        # ---------------- load gathered tensors ----------------
        # U_G[p = 16 ia + 4 ib + j, r, (b c)]: chunks are contiguous, so
        # the whole gather loads in one full-width DMA
        U_G = sb.tile([128, 3, BC], bf16, name="U_G")
        cc_v = cc_out[:].rearrange("(p r f) -> p r f", p=128, r=3, f=BC)
        nc.sync.dma_start(U_G[:, :, 0:512], cc_v[:, :, 0:512])
        nc.sync.dma_start(U_G[:, :, 512:], cc_v[:, :, 512:])

        # u_dot in [o, (a, ib)] layout; the [10, 8, 8] tile keeps the
        # (a, ib) dims non-collapsible so the DMA AP balances 1:1
        ud_t2 = sb.tile([OC, NCORES, 8], bf16, name="ud_t2")
        cc_r = cc_out[:].rearrange("(a x) -> a x", a=NCORES)
        nc.sync.dma_start(
            ud_t2[:, :, 0:ICL],
            cc_r[:, UD_OFF:UD_OFF + ICL * OC].rearrange(
                "a (o i) -> o a i", i=ICL, o=OC))
        ud_v = ud_t2[:, :, 0:ICL]

        # u_dot as exparg lhsT: udT[o, p = 16a + 4ib + j] = u_dot[i(p), o]
        udT = sb.tile([OC, NCORES, ICL, 4], bf16, name="udT")
        nc.vector.tensor_copy(
            udT[:],
            ud_t2[:, :, 0:ICL].rearrange("o a (i u) -> o a i u", i=ICL,
                                         u=1).broadcast_to(
                                             [OC, NCORES, ICL, 4]))

        # keep PE busy through the collective wait so routing iteration 0
        # starts at a ramped p-state; ~3us of dead matmuls on loaded data,
        # finished long before U_G can arrive
        uw_flat = uw_sb[:].rearrange("p i k f -> p (i k f)")
        with tc.tile_pool(name="psw", bufs=1, space="PSUM") as psw:
            wps = psw.tile([B, 512], f32, name="wps")
            for _d in range(7):
                nc.tensor.matmul(wps[:], uw_sb[:, 0, 0, 0:B],
                                 uw_flat[:, 0:512], start=True, stop=True)

        # ---------------- routing loop ----------------
        cgzb = sb.tile([128, 3 * OC], bf16, name="cgzb")
        cgz_f = sb.tile([128, 3 * OC], f32, name="cgz_f")

        with tc.tile_pool(name="ps2", bufs=2, space="PSUM") as ps2:
            for t in range(NUM_ROUTING):
                lhs = md_sb if t == 0 else cgzb
                # s~[o, (b c)] = sum_p cgz[p, o-slot] * U_G[p, r, (b c)];
                # two independent psum tiles so half 1's matmuls don't
                # stall on half 0's readers; abs/srow per half
                sjh = [ps2.tile([OC, 512], f32, name=f"sj{h}")
                       for h in range(2)]
                abs_scr = sb.tile([OC, BC], bf16, name="abs_scr", bufs=2)
                absr2 = sb.tile([OC, 2], f32, name="absr2", bufs=2)
                srow2 = sb.tile([OC, 2], f32, name="srow2", bufs=2)
                for h in range(2):
                    sl = slice(512 * h, 512 * (h + 1))
                    for r in range(KC):
                        nc.tensor.matmul(
                            sjh[h][:],
                            lhs[:, OC * r:OC * (r + 1)],
                            U_G[:, r, sl],
                            start=(r == 0), stop=(r == 2))
                    # scale=rcp folds the softmax norm into the abs accum
                    nc.scalar.activation(abs_scr[:, sl], sjh[h][:],
                                         ACT.Abs, scale=rcp[:],
                                         accum_out=absr2[:, h:h + 1])
                    if t < NUM_ROUTING - 1:
                        nc.vector.tensor_reduce(
                            srow2[:, h:h + 1], sjh[h][:], axis=AX.X,
                            op=OP.add)
                        if h == 1:
                            # srow~ h0+h1 and *rcp while Act does abs h1
                            srow = sb.tile([OC, 1], f32, name="srow",
                                           bufs=2)
                            nc.vector.tensor_tensor(srow[:], srow2[:, 0:1],
                                                    srow2[:, 1:2], OP.add)
                            rs = sb.tile([OC, 1], f32, name="rs", bufs=2)
                            nc.vector.tensor_scalar(rs[:], srow[:], rcp[:],
                                                    None, OP.mult)

                # n = sum_o rcp[o] * sum_f |s~| on all partitions via
                # ones-matmul (rcp already folded into the abs accums)
                absum = sb.tile([OC, 1], f32, name="absum", bufs=2)
                nc.vector.tensor_tensor(absum[:], absr2[:, 0:1],
                                        absr2[:, 1:2], OP.add)
                nbc = ps2.tile([OC, 1], f32, name="nbc")
                nc.tensor.matmul(nbc[:], ones_kk[:], absum[:],
                                 start=True, stop=True)
                dd = sb.tile([OC, 1], f32, name="dd", bufs=2)
                nc.vector.tensor_scalar(dd[:], nbc[:], nbc[:], 1.0,
                                        OP.mult, OP.add)
                rdd = sb.tile([OC, 1], f32, name="rdd", bufs=2)
                nc.vector.reciprocal_approx_fast(rdd[:], dd[:])

                if t < NUM_ROUTING - 1:
                    # w10 = n/(1+n^2) * rcp * srow~ in one op
                    w10 = sb.tile([OC, 1], f32, name="w10", bufs=2)
                    nc.vector.tensor_scalar(w10[:], rdd[:], nbc[:], rs[:],
                                            OP.mult, OP.mult)
                    W30 = sb.tile([OC, 3 * OC], bf16, name="W30", bufs=2)
                    nc.vector.tensor_scalar(W30[:],
                                            md_sb[0:OC, 3 * OC:6 * OC],
                                            w10[:], None, OP.mult)
                    # cgz *= exp(u_dot[i(p), o'] * w10[o'])
                    exparg = ps2.tile([128, 3 * OC], f32, name="exparg")
                    nc.tensor.matmul(exparg[:], udT[:], W30[:],
                                     start=True, stop=True)
                    expfac = sb.tile([128, 3 * OC], f32, name="expfac",
                                     bufs=2)
                    nc.scalar.activation(expfac[:], exparg[:], ACT.Exp)
                    prev = md_sb[:, 0:3 * OC] if t == 0 else cgz_f[:]
                    nc.vector.tensor_tensor(cgzb[:], prev, expfac[:],
                                            OP.mult)
                    if t < NUM_ROUTING - 2:
                        nc.vector.tensor_tensor(cgz_f[:], prev,
                                                expfac[:], OP.mult)

                    # Z for the next iteration (off critical path):
                    # b += u_dot * w10; rcp = 1/sum_i exp(b)
                    agree = sb.tile([OC, NCORES, ICL], f32, name="agree",
                                    bufs=2)
                    nc.vector.tensor_scalar(agree[:], ud_v, w10[:], None,
                                            OP.mult)
                    nc.vector.tensor_tensor(bt[:], bt[:], agree[:], OP.add)
                    ct_scr = sb.tile([OC, NCORES * ICL], f32, name="ct_scr",
                                     bufs=2)
                    esum = sb.tile([OC, 1], f32, name="esum", bufs=2)
                    nc.scalar.activation(
                        ct_scr[:], bt[:].rearrange("o a i -> o (a i)"),
                        ACT.Exp, accum_out=esum[:])
                    nc.vector.reciprocal_approx_fast(rcp[:], esum[:])
                else:
                    # v = s~ * (rcp * n / (1 + n^2)), stored per half
                    scale10 = sb.tile([OC, 1], f32, name="scale10")
                    nc.vector.tensor_scalar(scale10[:], rdd[:], nbc[:],
                                            rcp[:], OP.mult, OP.mult)
                    v_sb = sb.tile([OC, B, OCH], f32, name="v_sb")
                    v_view = v_out[:].rearrange("b o c -> o b c")
                    for h in range(2):
                        bs = slice(32 * h, 32 * (h + 1))
                        # h0 on DVE, h1 on Act: the halves run in parallel
                        vh = v_sb[:, bs, :].rearrange("o b c -> o (b c)")
                        if h == 0:
                            nc.vector.tensor_scalar(vh, sjh[h][:],
                                                    scale10[:], None,
                                                    OP.mult)
                        else:
                            nc.scalar.activation(vh, sjh[h][:], ACT.Copy,
                                                 scale=scale10[:])
                        nc.sync.dma_start(v_view[:, bs, :],
                                          v_sb[:, bs, :])

    with tile.TileContext(nc) as tc:
        with (
            tc.tile_pool(name="sb", bufs=1) as sb,
            tc.tile_pool(name="dram", bufs=1, space="DRAM") as dram,
        ):
            for _rep in range(reps):
                emit(tc, sb, dram)

    nc.compile()
    _CACHE[key] = nc
    return nc


def make_in_maps(u, W):
    import ml_dtypes
    bf16 = ml_dtypes.bfloat16

    u = np.ascontiguousarray(np.asarray(u, dtype=np.float32))
    W = np.ascontiguousarray(np.asarray(W, dtype=np.float32))
    # [B, IC, D] / [IC, D, OC*OCH] -> per-core [128, i:4, k:3, 64+160]
    uw = np.zeros((IC, KC, 128, B + OC * OCH), dtype=np.float32)
    uw.reshape(IC, DP, B + OC * OCH)[:, :D, 0:B] = (
        u.reshape(B, IC, D).transpose(1, 2, 0))
    uw.reshape(IC, DP, B + OC * OCH)[:, :D, B:] = W.reshape(IC, D, OC * OCH)
    uw = np.ascontiguousarray(uw.transpose(2, 0, 1, 3)).astype(bf16)

    # md cols 0:30 = mask[p, 10r + o] = [o == 4r + (p % 4)];
    # md cols 30:60 (parts 0:10) = diag[o, 10r + o'] = [o == o']
    p = np.arange(128)
    md = np.zeros((128, 2 * 3 * OC), dtype=np.float32)
    for r in range(KC):
        for o in range(OC):
            md[:, OC * r + o] = (o == 4 * r + (p % 4))
    md[0:OC, 3 * OC:] = np.tile(np.eye(OC, dtype=np.float32), (1, KC))
    md = md.astype(bf16)

    return [
        {
            "uw_t": np.ascontiguousarray(uw[:, ICL * r: ICL * (r + 1)]),
            "md": md,
        }
        for r in range(NCORES)
    ]


def get_runner(nc):
    """Build (once) a jitted 8-core executor for the compiled program.

    Mirrors bass2jax.run_bass_via_pjrt's multi-core path but caches the
    jitted callable so repeated kernel() calls skip retracing.
    """
    if "runner" in _CACHE and _CACHE["runner"][0] is nc:
        return _CACHE["runner"][1]

    import jax
    from jax.sharding import Mesh, PartitionSpec
    from jax.experimental.shard_map import shard_map
    from concourse import mybir
    from concourse.bass2jax import (_bass_exec_p, install_neuronx_cc_hook,
                                    partition_id_tensor)

    install_neuronx_cc_hook()
    partition_name = (nc.partition_id_tensor.name
                      if nc.partition_id_tensor else None)
    in_names, out_names, out_avals, zero_outs = [], [], [], []
    for alloc in nc.m.functions[0].allocations:
        if not isinstance(alloc, mybir.MemoryLocationSet):
            continue
        name = alloc.memorylocations[0].name
        if alloc.kind == "ExternalInput":
            if name != partition_name:
                in_names.append(name)
        elif alloc.kind == "ExternalOutput":
            out_names.append(name)
            shape = tuple(alloc.tensor_shape)
            dtype = mybir.dt.np(alloc.dtype)
            out_avals.append(jax.core.ShapedArray(shape, dtype))
            zero_outs.append(np.zeros(shape, dtype))
    n_params = len(in_names)
    n_outs = len(out_avals)
    all_in_names = list(in_names) + list(out_names)
    if partition_name is not None:
        all_in_names.append(partition_name)

    def _body(*args):
        operands = list(args)
        if partition_name is not None:
            operands.append(partition_id_tensor())
        return tuple(_bass_exec_p.bind(
            *operands,
            out_avals=tuple(out_avals),
            in_names=tuple(all_in_names),
            out_names=tuple(out_names),
            lowering_input_output_aliases=(),
            sim_require_finite=True,
            sim_require_nnan=True,
            nc=nc,
        ))

    devices = jax.devices()[:NCORES]
    mesh = Mesh(np.asarray(devices), ("core",))
    sharded = jax.jit(
        shard_map(_body, mesh=mesh,
                  in_specs=(PartitionSpec("core"),) * (n_params + n_outs),
                  out_specs=(PartitionSpec("core"),) * n_outs,
                  check_rep=False),
        donate_argnums=tuple(range(n_params, n_params + n_outs)),
        keep_unused=True)

    def run(in_maps):
        concat_in = [
            np.concatenate([np.asarray(m[nm]) for m in in_maps], axis=0)
            for nm in in_names
        ]
        concat_zeros = [np.zeros((NCORES * z.shape[0], *z.shape[1:]), z.dtype)
                        for z in zero_outs]
        outs = sharded(*concat_in, *concat_zeros)
        jax.block_until_ready(outs)
        return {
            nm: np.asarray(outs[i]).reshape(NCORES, *out_avals[i].shape)
            for i, nm in enumerate(out_names)
        }

    _CACHE["runner"] = (nc, run)
    return run


def kernel(u, W):
    nc = build()
    run = get_runner(nc)
    out = run(make_in_maps(u, W))
    return np.asarray(out["v"][0], dtype=np.float32).reshape(B, OC, OCH)



# revision 15
# speedup vs baseline: 62.6752x; 62.6752x over previous
"""DigitCapsules routing kernel for 8 Trainium2 NeuronCores.

Strategy: shard the in_capsule dimension (32 -> 4 per core) so each core
reads only its slice of W (the dominant tensor) and u.  Each core computes
its u_sum slice [B, 4, OC, OCH] with TensorE matmuls (bf16 inputs, fp32
accumulate) and derives its u_dot slice [4, OC] directly from u_sum
(u_dot[i,o] = sum_{b,c} u_sum[b,i,o,c]) with one reduce + one ones-matmul.
One AllGather shares all slices and every core runs the tiny 3-iteration
dynamic-routing loop redundantly on the full u_sum.  Core 0's output is
returned.

Input packing: the contraction dim D=288 splits into k-chunks 128+128+32;
the 32-row tails of the 4 local capsules share one 128-partition tile
(uwk2), so the load is 505KB instead of a zero-padded 688KB.

Layouts: the gathered u_sum lives as U_G[p, r, (b c)] with the partition
index p = 16*ia + 4*ib + j encoding (rank ia, local capsule ib, o-group j)
and o = 4*r + j.  The per-o mixing weights are kept directly in the masked
lhsT layout cgz[p, 10*r + o] = c~[i(p), o] * [o == 4*r + j(p)] (c~ =
unnormalized softmax numerator) and updated multiplicatively each routing
iteration: cgz *= exp(u_dot[i(p), o'] * w[o']), where the rank-1 argument
is produced by a single k=10 matmul.  1/sumexp rides the post-matmul
per-partition scalars.

PSUM pools are opened once (outside the per-rep emit) so back-to-back
pipeline reps don't pay pool-close engine drains; dead matmuls on loaded
data keep the PE p-state ramped across the collective and scalar-chain
gaps.

Self-contained: hardcodes all shapes; only imports installed packages.
"""

import numpy as np

# problem shapes
B = 64
IC = 32
ICH = 8
WID = 6
HEI = 6
D = ICH * WID * HEI          # 288
KC = 3                       # k-chunks: 128 + 128 + 32
OC = 10
OCH = 16
BC = B * OCH                 # 1024
NUM_ROUTING = 3
NCORES = 8
ICL = IC // NCORES           # 4 local in-capsules per core
FW = B + OC * OCH            # 224 packed columns (u batch | W)

# AllGather chunk layout per rank (bf16): (ib:4, jr:12, b:64, c:16) with
# jr = 3*j + r and o = 4*r + j.  Slots with o >= 10 are zero padding that
# the cgz mask ignores, so u_dot (o, i) is stashed inside the (ib=0, jr=8)
# pad slot and the whole chunk stays contiguous (one-DMA gather far side).
USZ = 4 * 4 * 3 * B * OCH    # 49152 bf16 -> 98304 B, 32-aligned
CHUNK = USZ
UD_OFF = 2 * 3 * B * OCH + 2 * B * OCH  # 8192, slot (ib=0, j=2, r=2)

_CACHE = {}


def build(reps=1, single_core=False, no_cc=False, warm0=10, warm12=4):
    """Build + compile the SPMD Bass program (cached per config).

    reps > 1 repeats the complete pipeline sequentially; used only for
    wall-clock differencing to estimate the on-device execution time.
    single_core=True swaps the AllGather for local DMA copies (same bytes
    moved) so the collective-free program can run under TimelineSim.
    no_cc=True keeps 8 devices but replaces the AllGather with local
    copies (wrong results; used to isolate the collective's HW cost).
    """
    key = ("nc", reps, single_core, no_cc, warm0, warm12)
    if key in _CACHE:
        return _CACHE[key]

    import concourse.bass as bass  # noqa: F401
    import concourse.mybir as mybir
    from concourse import tile, bacc

    f32 = mybir.dt.float32
    bf16 = mybir.dt.bfloat16
    AX = mybir.AxisListType
    OP = mybir.AluOpType
    ACT = mybir.ActivationFunctionType

    local_cc = single_core or no_cc
    nc = bacc.Bacc("TRN2", target_bir_lowering=False, debug=False,
                   num_devices=1 if single_core else NCORES)

    # inputs pre-arranged on the host so every load is a contiguous DMA
    uw_in = nc.dram_tensor("uw_t", [128, ICL, 2, FW], bf16,
                           kind="ExternalInput")
    # k2 tails: capsule i at partitions 32*(i%2), column block i//2
    # (matmul base partition must be 0/32/64, so 4x32 can't stack flat)
    uwk2_in = nc.dram_tensor("uwk2", [64, 2, FW], bf16,
                             kind="ExternalInput")
    # md packs maskm (cols 0:30) and diagm (parts 0:10, cols 30:60)
    md_in = nc.dram_tensor("md", [128, 2 * 3 * OC], bf16,
                           kind="ExternalInput")
    v_out = nc.dram_tensor("v", [B, OC, OCH], f32, kind="ExternalOutput")

    def emit(tc, sb, dram, ps1, psw, ps2):
        # ---------------- phase 1: local u_sum + u_dot ----------------
        uw_sb = sb.tile([128, ICL, 2, FW], bf16, name="uw_sb", bufs=2)
        nc.sync.dma_start(uw_sb[:, 0:2], uw_in[:, 0:2])
        uwk2_sb = sb.tile([64, 2, FW], bf16, name="uwk2_sb", bufs=2)
        nc.scalar.dma_start(uwk2_sb[:], uwk2_in[:])
        nc.sync.dma_start(uw_sb[:, 2:4], uw_in[:, 2:4])
        md_sb = sb.tile([128, 2 * 3 * OC], bf16, name="md_sb", bufs=2)
        nc.sync.dma_start(md_sb[:], md_in[:])

        # constants + state built on-chip, off the critical path
        ones_kk = sb.tile([OC, OC], f32, name="ones_kk", bufs=2)
        nc.vector.memset(ones_kk[:], 1.0)
        ones64 = sb.tile([B, 1], f32, name="ones64", bufs=2)
        nc.vector.memset(ones64[:], 1.0)
        # dummy activation pulls LoadActFuncSet under the input-DMA wait
        warm = sb.tile([1, 1], f32, name="warm", bufs=2)
        nc.scalar.activation(warm[:], ones_kk[0:1, 0:1], ACT.Abs)
        bt = sb.tile([OC, NCORES, ICL], f32, name="bt", bufs=2)
        nc.vector.memset(bt[:], 0.0)
        rcp = sb.tile([OC, 1], f32, name="rcp", bufs=2)
        nc.vector.memset(rcp[:], 1.0 / 32.0)  # softmax(0): Z_o = 32

        # local u_sum in chunk layout (b, ib, j, r, c); (j>=2, r=2) zeroed
        s_all = sb.tile([B, ICL, 4, 3, OCH], bf16, name="s_all", bufs=2)
        nc.gpsimd.memset(s_all[:, :, 2:4, 2, :], 0.0)

        q4f = sb.tile([B, OC, ICL], f32, name="q4f", bufs=2)
        cc_in = dram.tile([CHUNK], bf16, name="cc_in", bufs=2)
        cc_sv = cc_in[0:USZ].rearrange("(ib g b c) -> b ib g c",
                                       ib=ICL, g=12, b=B, c=OCH)
        p1s = []
        for pair in ((0, 1), (2, 3)):
            for i in pair:
                p1 = ps1.tile([B, OC, OCH], f32, name="p1")
                p1s.append(p1)
                for k in range(KC):
                    if k < 2:
                        lhsT = uw_sb[:, i, k, 0:B]
                        rhs = uw_sb[:, i, k, B:]
                    else:
                        pb = 32 * (i % 2)
                        lhsT = uwk2_sb[pb:pb + 32, i // 2, 0:B]
                        rhs = uwk2_sb[pb:pb + 32, i // 2, B:]
                    nc.tensor.matmul(p1[:], lhsT, rhs,
                                     start=(k == 0), stop=(k == KC - 1))
                # u_sum slice -> s_all columns (j, r) with o = 4r + j
                nc.vector.tensor_copy(
                    s_all[:, i].rearrange("b j r c -> b r j c")[:, 0:2],
                    p1[:, 0:8, :].rearrange("b (r j) c -> b r j c",
                                            r=2, j=4))
                nc.scalar.activation(s_all[:, i, 0:2, 2, :],
                                     p1[:, 8:10, :], ACT.Copy)
            # store the finished pair before the q4 reduces so the
            # chunk (and the collective behind it) leaves earlier
            i0 = pair[0]
            nc.sync.dma_start(
                cc_sv[:, i0:i0 + 2],
                s_all[:, i0:i0 + 2].rearrange(
                    "b ib j r c -> b ib (j r) c"))
            for i in pair:
                # q4[b, o, i] = sum_c u_sum (feeds u_dot)
                nc.vector.tensor_reduce(q4f[:, :, i], p1s[i][:],
                                        axis=AX.X, op=OP.add)
        # u_dot row (o, i): sum_b q4 via fp32 ones-matmul
        udp = ps2.tile([1, OC * ICL], f32, name="udp")
        nc.tensor.matmul(udp[:], ones64[:],
                         q4f[:].rearrange("b o i -> b (o i)"),
                         start=True, stop=True)
        ud_sb = sb.tile([1, OC * ICL], bf16, name="ud_sb", bufs=2)
        nc.scalar.activation(ud_sb[:], udp[:], ACT.Copy)
        nc.scalar.dma_start(cc_in[UD_OFF:UD_OFF + ICL * OC], ud_sb[:])

        # ---------------- AllGather ----------------
        cc_out = dram.tile([NCORES * CHUNK], bf16,
                           addr_space="Local" if local_cc else "Shared",
                           name="cc_out", bufs=2)
        if local_cc:
            for ia in range(NCORES):
                nc.sync.dma_start(
                    cc_out[ia * CHUNK:(ia + 1) * CHUNK], cc_in[:])
        else:
            nc.gpsimd.collective_compute(
                "AllGather", OP.bypass,
                replica_groups=[list(range(NCORES))],
                ins=[cc_in[:]], outs=[cc_out[:]],
            )

        # ---------------- load gathered tensors ----------------
        # U_G[p = 16 ia + 4 ib + j, r, (b c)]: chunks are contiguous; the
        # two halves issue from different queues (SP / Act)
        U_G = sb.tile([128, 3, BC], bf16, name="U_G", bufs=2)
        cc_v = cc_out[:].rearrange("(p r f) -> p r f", p=128, r=3, f=BC)
        nc.sync.dma_start(U_G[:, :, 0:512], cc_v[:, :, 0:512])
        nc.scalar.dma_start(U_G[:, :, 512:], cc_v[:, :, 512:])

        # u_dot in [o, (a, ib)] layout; the [10, 8, 8] tile keeps the
        # (a, ib) dims non-collapsible so the DMA AP balances 1:1
        ud_t2 = sb.tile([OC, NCORES, 8], bf16, name="ud_t2", bufs=2)
        cc_r = cc_out[:].rearrange("(a x) -> a x", a=NCORES)
        nc.gpsimd.dma_start(
            ud_t2[:, :, 0:ICL],
            cc_r[:, UD_OFF:UD_OFF + ICL * OC].rearrange(
                "a (o i) -> o a i", i=ICL, o=OC))
        ud_v = ud_t2[:, :, 0:ICL]

        # u_dot as exparg lhsT: udT[o, p = 16a + 4ib + j] = u_dot[i(p), o]
        udT = sb.tile([OC, NCORES, ICL, 4], bf16, name="udT", bufs=2)
        nc.vector.tensor_copy(
            udT[:],
            ud_t2[:, :, 0:ICL].rearrange("o a (i u) -> o a i u", i=ICL,
                                         u=1).broadcast_to(
                                             [OC, NCORES, ICL, 4]))

        # keep PE busy through the collective wait so routing iteration 0
        # starts at a ramped p-state; back-to-back dead matmuls on loaded
        # data sized to roughly cover the AllGather window
        uw_flat = uw_sb[:].rearrange("p i k f -> p (i k f)")
        wps = psw.tile([B, 512], f32, name="wps")
        for _d in range(warm0):
            nc.tensor.matmul(wps[:], uw_sb[:, 0, 0, 0:B],
                             uw_flat[:, 0:512], start=True, stop=True)

        # ---------------- routing loop ----------------
        cgzb = sb.tile([128, 3 * OC], bf16, name="cgzb", bufs=2)
        cgz_f = sb.tile([128, 3 * OC], f32, name="cgz_f", bufs=2)

        for t in range(NUM_ROUTING):
            lhs = md_sb if t == 0 else cgzb
            # s~[o, (b c)] = sum_p cgz[p, o-slot] * U_G[p, r, (b c)];
            # abs-accum h0 on Act, h1 on DVE; srow halves on Pool
            sjh = [ps2.tile([OC, 512], f32, name=f"sj{h}")
                   for h in range(2)]
            abs_scr = sb.tile([OC, 512], bf16, name="abs_scr", bufs=2)
            absr2 = sb.tile([OC, 2], f32, name="absr2", bufs=2)
            srow2 = sb.tile([OC, 2], f32, name="srow2", bufs=2)
            for h in range(2):
                sl = slice(512 * h, 512 * (h + 1))
                for r in range(KC):
                    nc.tensor.matmul(
                        sjh[h][:],
                        lhs[:, OC * r:OC * (r + 1)],
                        U_G[:, r, sl],
                        start=(r == 0), stop=(r == 2))
                # abs-sums: h0 on Act (scale=rcp folded), h1 on DVE
                # (rcp folded at the absum combine)
                if h == 0:
                    nc.scalar.activation(abs_scr[:], sjh[h][:],
                                         ACT.Abs, scale=rcp[:],
                                         accum_out=absr2[:, h:h + 1])
                else:
                    nc.vector.tensor_reduce(absr2[:, h:h + 1], sjh[h][:],
                                            axis=AX.X, op=OP.add,
                                            apply_absolute_value=True)
                if t < NUM_ROUTING - 1:
                    nc.vector.tensor_reduce(srow2[:, h:h + 1], sjh[h][:],
                                            axis=AX.X, op=OP.add)
                    if h == 1:
                        # srow~ h0+h1 and *rcp while the abs finishes
                        srow = sb.tile([OC, 1], f32, name="srow",
                                       bufs=2)
                        nc.vector.tensor_tensor(srow[:], srow2[:, 0:1],
                                                srow2[:, 1:2], OP.add)
                        rs = sb.tile([OC, 1], f32, name="rs", bufs=2)
                        nc.vector.tensor_scalar(rs[:], srow[:], rcp[:],
                                                None, OP.mult)

            # n = sum_o rcp[o] * sum_f |s~| on all partitions via
            # ones-matmul (h0 pre-scaled on Act; h1's rcp folded here)
            absum = sb.tile([OC, 1], f32, name="absum", bufs=2)
            nc.vector.tensor_scalar(absum[:], absr2[:, 1:2], rcp[:],
                                    absr2[:, 0:1], OP.mult, OP.add)
            nbc = ps2.tile([OC, 1], f32, name="nbc")
            nc.tensor.matmul(nbc[:], ones_kk[:], absum[:],
                             start=True, stop=True)
            dd = sb.tile([OC, 1], f32, name="dd", bufs=2)
            nc.vector.tensor_scalar(dd[:], nbc[:], nbc[:], 1.0,
                                    OP.mult, OP.add)
            rdd = sb.tile([OC, 1], f32, name="rdd", bufs=2)
            nc.vector.reciprocal_approx_fast(rdd[:], dd[:])

            if t < NUM_ROUTING - 1:
                # w10 = n/(1+n^2) * rcp * srow~ in one op
                w10 = sb.tile([OC, 1], f32, name="w10", bufs=2)
                nc.vector.tensor_scalar(w10[:], rdd[:], nbc[:], rs[:],
                                        OP.mult, OP.mult)
                W30 = sb.tile([OC, 3 * OC], bf16, name="W30", bufs=2)
                nc.vector.tensor_scalar(W30[:],
                                        md_sb[0:OC, 3 * OC:6 * OC],
                                        w10[:], None, OP.mult)
                # cgz *= exp(u_dot[i(p), o'] * w10[o'])
                exparg = ps2.tile([128, 3 * OC], f32, name="exparg")
                nc.tensor.matmul(exparg[:], udT[:], W30[:],
                                 start=True, stop=True)
                expfac = sb.tile([128, 3 * OC], f32, name="expfac",
                                 bufs=2)
                nc.scalar.activation(expfac[:], exparg[:], ACT.Exp)
                prev = md_sb[:, 0:3 * OC] if t == 0 else cgz_f[:]
                nc.vector.tensor_tensor(cgzb[:], prev, expfac[:],
                                        OP.mult)
                if t < NUM_ROUTING - 2:
                    nc.vector.tensor_tensor(cgz_f[:], prev,
                                            expfac[:], OP.mult)

                # Z for the next iteration (off critical path):
                # b += u_dot * w10; rcp = 1/sum_i exp(b)
                agree = sb.tile([OC, NCORES, ICL], f32, name="agree",
                                bufs=2)
                nc.vector.tensor_scalar(agree[:], ud_v, w10[:], None,
                                        OP.mult)
                nc.vector.tensor_tensor(bt[:], bt[:], agree[:], OP.add)
                ct_scr = sb.tile([OC, NCORES * ICL], f32, name="ct_scr",
                                 bufs=2)
                esum = sb.tile([OC, 1], f32, name="esum", bufs=2)
                nc.scalar.activation(
                    ct_scr[:], bt[:].rearrange("o a i -> o (a i)"),
                    ACT.Exp, accum_out=esum[:])
                nc.vector.reciprocal_approx_fast(rcp[:], esum[:])
                # dead matmuls bridge the scalar-chain gap so the next
                # iteration's matmuls stay at a ramped p-state
                for _d in range(warm12):
                    nc.tensor.matmul(wps[:], uw_sb[:, 0, 0, 0:B],
                                     uw_flat[:, 0:512],
                                     start=True, stop=True)
            else:
                # v = s~ * (rcp * n / (1 + n^2)), stored per half
                scale10 = sb.tile([OC, 1], f32, name="scale10", bufs=2)
                nc.vector.tensor_scalar(scale10[:], rdd[:], nbc[:],
                                        rcp[:], OP.mult, OP.mult)
                v_sb = sb.tile([OC, B, OCH], f32, name="v_sb", bufs=2)
                v_view = v_out[:].rearrange("b o c -> o b c")
                for h in range(2):
                    bs = slice(32 * h, 32 * (h + 1))
                    # h0 on DVE, h1 on Act: the halves run in parallel
                    vh = v_sb[:, bs, :].rearrange("o b c -> o (b c)")
                    if h == 0:
                        nc.vector.tensor_scalar(vh, sjh[h][:],
                                                scale10[:], None,
                                                OP.mult)
                    else:
                        nc.scalar.activation(vh, sjh[h][:], ACT.Copy,
                                             scale=scale10[:])
                    nc.sync.dma_start(v_view[:, bs, :],
                                      v_sb[:, bs, :])

    with tile.TileContext(nc) as tc:
        with (
            tc.tile_pool(name="sb", bufs=1) as sb,
            tc.tile_pool(name="dram", bufs=1, space="DRAM") as dram,
            tc.tile_pool(name="ps1", bufs=2, space="PSUM") as ps1,
            tc.tile_pool(name="psw", bufs=1, space="PSUM") as psw,
            tc.tile_pool(name="ps2", bufs=1, space="PSUM") as ps2,
        ):
            for _rep in range(reps):
                emit(tc, sb, dram, ps1, psw, ps2)

    nc.compile()
    _CACHE[key] = nc
    return nc


def make_in_maps(u, W):
    import ml_dtypes
    bf16 = ml_dtypes.bfloat16

    u = np.ascontiguousarray(np.asarray(u, dtype=np.float32))
    W = np.ascontiguousarray(np.asarray(W, dtype=np.float32))
    # [B, IC, D] / [IC, D, OC*OCH] -> packed per-core chunks:
    # uw [128, i:4, k:2, 224] holds D-rows 0:256; uwk2 [128, 224] holds
    # rows 256:288 of the 4 local capsules stacked on partitions.
    uwd = np.zeros((IC, D, FW), dtype=np.float32)
    uwd[:, :, 0:B] = u.reshape(B, IC, D).transpose(1, 2, 0)
    uwd[:, :, B:] = W.reshape(IC, D, OC * OCH)
    uw2 = np.ascontiguousarray(
        uwd[:, 0:256].reshape(IC, 2, 128, FW).transpose(2, 0, 1, 3)
    ).astype(bf16)                       # [128, IC, 2, 224]
    # tails [IC, 32, FW] -> per core [64, 2, FW]: capsule i%2 on
    # partition half, i//2 on the column block
    uk2 = np.ascontiguousarray(
        uwd[:, 256:288].reshape(NCORES, 2, 2, 32, FW)  # (core, i//2, i%2, p, f)
        .transpose(0, 2, 3, 1, 4)                      # (core, i%2, p, i//2, f)
        .reshape(NCORES, 64, 2, FW)
    ).astype(bf16)                       # [8, 64, 2, 224]

    # md cols 0:30 = mask[p, 10r + o] = [o == 4r + (p % 4)];
    # md cols 30:60 (parts 0:10) = diag[o, 10r + o'] = [o == o']
    p = np.arange(128)
    md = np.zeros((128, 2 * 3 * OC), dtype=np.float32)
    for r in range(KC):
        for o in range(OC):
            md[:, OC * r + o] = (o == 4 * r + (p % 4))
    md[0:OC, 3 * OC:] = np.tile(np.eye(OC, dtype=np.float32), (1, KC))
    md = md.astype(bf16)

    return [
        {
            "uw_t": np.ascontiguousarray(uw2[:, ICL * r: ICL * (r + 1)]),
            "uwk2": uk2[r],
            "md": md,
        }
        for r in range(NCORES)
    ]


def get_runner(nc):
    """Build (once) a jitted 8-core executor for the compiled program.

    Mirrors bass2jax.run_bass_via_pjrt's multi-core path but caches the
    jitted callable so repeated kernel() calls skip retracing.
    """
    if "runner" in _CACHE and _CACHE["runner"][0] is nc:
        return _CACHE["runner"][1]

    import jax
    from jax.sharding import Mesh, PartitionSpec
    from jax.experimental.shard_map import shard_map
    from concourse import mybir
    from concourse.bass2jax import (_bass_exec_p, install_neuronx_cc_hook,
                                    partition_id_tensor)

    install_neuronx_cc_hook()
    partition_name = (nc.partition_id_tensor.name
                      if nc.partition_id_tensor else None)
    in_names, out_names, out_avals, zero_outs = [], [], [], []
    for alloc in nc.m.functions[0].allocations:
        if not isinstance(alloc, mybir.MemoryLocationSet):
            continue
        name = alloc.memorylocations[0].name
        if alloc.kind == "ExternalInput":
            if name != partition_name:
                in_names.append(name)
        elif alloc.kind == "ExternalOutput":
            out_names.append(name)
            shape = tuple(alloc.tensor_shape)
            dtype = mybir.dt.np(alloc.dtype)
            out_avals.append(jax.core.ShapedArray(shape, dtype))
            zero_outs.append(np.zeros(shape, dtype))
    n_params = len(in_names)
    n_outs = len(out_avals)
    all_in_names = list(in_names) + list(out_names)
    if partition_name is not None:
        all_in_names.append(partition_name)

    def _body(*args):
        operands = list(args)
        if partition_name is not None:
            operands.append(partition_id_tensor())
        return tuple(_bass_exec_p.bind(
            *operands,
            out_avals=tuple(out_avals),
            in_names=tuple(all_in_names),
            out_names=tuple(out_names),
            lowering_input_output_aliases=(),
            sim_require_finite=True,
            sim_require_nnan=True,
            nc=nc,
        ))

    devices = jax.devices()[:NCORES]
    mesh = Mesh(np.asarray(devices), ("core",))
    sharded = jax.jit(
        shard_map(_body, mesh=mesh,
                  in_specs=(PartitionSpec("core"),) * (n_params + n_outs),
                  out_specs=(PartitionSpec("core"),) * n_outs,
                  check_rep=False),
        donate_argnums=tuple(range(n_params, n_params + n_outs)),
        keep_unused=True)

    def run(in_maps):
        concat_in = [
            np.concatenate([np.asarray(m[nm]) for m in in_maps], axis=0)
            for nm in in_names
        ]
        concat_zeros = [np.zeros((NCORES * z.shape[0], *z.shape[1:]), z.dtype)
                        for z in zero_outs]
        outs = sharded(*concat_in, *concat_zeros)
        jax.block_until_ready(outs)
        return {
            nm: np.asarray(outs[i]).reshape(NCORES, *out_avals[i].shape)
            for i, nm in enumerate(out_names)
        }

    _CACHE["runner"] = (nc, run)
    return run


def kernel(u, W):
    nc = build()
    run = get_runner(nc)
    out = run(make_in_maps(u, W))
    return np.asarray(out["v"][0], dtype=np.float32).reshape(B, OC, OCH)
